# revision 12
# baseline (speedup 1.0000x reference)
"""Trainium2 Bass kernel for nn_AdaptiveWaveletLayer.

Data-parallel over batch B across 8 NeuronCores (no collectives).
Per core: 12 graphs (t slices), each: masked-softmax attention over a
512x512 score matrix built from rank-1 terms, then 3 rounds of
U @ V message passing with all scalar coefficient algebra folded on host.

Device layout ((j,i) = transposed attention matrix, j on partitions):
  E'[j,i] = f1[i] + f2[j]      built by a K=4 fp16 hi/lo matmul into PSUM
  L = Lrelu(E', alpha=0.2)     ACT, PSUM->SBUF fp16
  Lm = L + Bmask (additive -30000 on masked entries)  DVE fp16 2x
  Eh = exp(Lm + bias_c)        ACT (per-graph range-shift bias), fp16
  G  = Eh * relu(adj)^T        DVE fp16 2x
  d[i] = sum_j Eh  (ones column folded into first matmul's rhs)
  s[i] = sum_j G   (ones-vector matmuls)
  W_k = Eh^T-contract matmuls; V_k = r * W_k  (r = 1/d)
  OUT = wx*x + w1*V1 + w2*V2 + w3*V3 (per-node affine weights in rowsum)
"""

import sys

if "/opt/trn_rl_repo" not in sys.path:
    sys.path.insert(0, "/opt/trn_rl_repo")

import numpy as np

B, N, T, C = 8, 512, 12, 64
P = 128
JT = N // P  # 4
HOP = 3
LEAKY = 0.2
MASK_NEG = -30000.0


def _sigmoid(x):
    return 1.0 / (1.0 + np.exp(-x))


def _build_bass(coefs, reps=None):
    """Build the single-core Bass graph. coefs: dict of python-float immediates."""
    from concourse import bacc, mybir
    from concourse.tile import TileContext

    f16 = mybir.dt.float16
    f32 = mybir.dt.float32
    Lrelu = mybir.ActivationFunctionType.Prelu
    Exp = mybir.ActivationFunctionType.Exp
    mult = mybir.AluOpType.mult
    add = mybir.AluOpType.add

    nc = bacc.Bacc()
    ep_d = nc.declare_dram_parameter("epack", [T, 8, 512], f16, isOutput=False)
    xp_d = nc.declare_dram_parameter("xp", [T, P, JT, 66], f16, isOutput=False)
    ad_d = nc.declare_dram_parameter("adjpack", [3, P, JT, 512], f16, isOutput=False)
    out_d = nc.declare_dram_parameter("out", [T, P, JT, C], f32, isOutput=True)

    Ax, Bx = coefs["Ax"], coefs["Bx"]
    Aw = [coefs["A1"], coefs["A2"], coefs["A3"]]
    Bw = [coefs["B1"], coefs["B2"], coefs["B3"]]

    with TileContext(nc) as tc:
        with (
            tc.tile_pool(name="const", bufs=1) as constp,
            tc.tile_pool(name="gin", bufs=4) as gin,
            tc.tile_pool(name="gbig", bufs=3) as gbig,
            tc.tile_pool(name="gsm", bufs=2) as gsm,
            tc.tile_pool(name="gout", bufs=3) as gout,
            tc.tile_pool(name="eps", bufs=1, space="PSUM") as epsp,
            tc.tile_pool(name="wps", bufs=2, space="PSUM") as wpsp,
            tc.tile_pool(name="sps", bufs=2, space="PSUM") as spsp,
        ):
            # ---- shared constants ----
            am_sb = constp.tile([P, JT, 512], f16, tag="am")
            bm_sb = constp.tile([P, JT, 512], f16, tag="bm")
            id_sb = constp.tile([P, 128], f16, tag="idm")
            ones_sb = constp.tile([P, 1], f16, tag="ones")
            nc.sync.dma_start(am_sb[:], ad_d[0])
            nc.sync.dma_start(bm_sb[:], ad_d[1])
            nc.sync.dma_start(id_sb[:], ad_d[2, :, 0, 0:128])
            nc.vector.memset(ones_sb[:], 1.0)

            import contextlib

            loop_cm = tc.For_i(0, reps, 1) if reps else contextlib.nullcontext()
            with loop_cm:
                _body_graphs(nc, tc, locals())

    nc.finalize()
    return nc


def _body_graphs(nc, tc, env):
    from concourse import mybir

    f16 = mybir.dt.float16
    f32 = mybir.dt.float32
    Lrelu = mybir.ActivationFunctionType.Prelu
    Exp = mybir.ActivationFunctionType.Exp
    mult = mybir.AluOpType.mult
    add = mybir.AluOpType.add
    gin, gbig, gsm, gout = env["gin"], env["gbig"], env["gsm"], env["gout"]
    epsp, wpsp, spsp = env["epsp"], env["wpsp"], env["spsp"]
    am_sb, bm_sb, ones_sb, id_sb = env["am_sb"], env["bm_sb"], env["ones_sb"], env["id_sb"]
    ep_d, xp_d, out_d = env["ep_d"], env["xp_d"], env["out_d"]
    Ax, Bx, Aw, Bw = env["Ax"], env["Bx"], env["Aw"], env["Bw"]
    if True:
            for t in range(T):
                # ---- per-graph inputs ----
                elhs = gin.tile([4, 512], f16, tag="elhs")
                erhs = gin.tile([4, 512], f16, tag="erhs")
                xp_sb = gin.tile([P, JT, 66], f16, tag="xp")
                nc.sync.dma_start(elhs[:], ep_d[t, 0:4, :])
                nc.sync.dma_start(erhs[:], ep_d[t, 4:8, :])
                nc.sync.dma_start(xp_sb[:], xp_d[t])

                # ---- E' = f1[i] + f2[j]  (K=4 fp16 hi/lo) ----
                e_ps = epsp.tile([P, JT, 512], f32, tag="eps")
                for jt in range(JT):
                    nc.tensor.matmul(
                        e_ps[:, jt, :],
                        elhs[:, jt * P : (jt + 1) * P],
                        erhs[:],
                        start=True,
                        stop=False,
                    )
                    nc.tensor.matmul(
                        e_ps[:, jt, :],
                        id_sb[:],
                        bm_sb[:, jt, :],
                        start=False,
                        stop=True,
                    )

                # ---- leaky relu (mask already folded into E) ----
                l_sb = gbig.tile([P, JT, 512], f16, tag="lsb")
                nc.scalar.activation(l_sb[:], e_ps[:], Lrelu, alpha=LEAKY)

                # ---- exp with per-graph range shift ----
                eh_sb = gbig.tile([P, JT, 512], f16, tag="ehsb")
                nc.scalar.activation(
                    eh_sb[:], l_sb[:], Exp, bias=xp_sb[:, 0, 65:66], scale=1.0
                )

                # ---- G = Eh * relu(adj)^T ----
                g_sb = gbig.tile([P, JT, 512], f16, tag="gsb")
                nc.vector.tensor_mul(g_sb[:], eh_sb[:], am_sb[:])

                # ---- s[i] = sum_j G ----
                s_ps = spsp.tile([P, JT], f32, tag="sps")
                for it in range(JT):
                    for jc in range(JT):
                        nc.tensor.matmul(
                            s_ps[:, it : it + 1],
                            g_sb[:, jc, it * P : (it + 1) * P],
                            ones_sb[:],
                            start=(jc == 0),
                            stop=(jc == JT - 1),
                        )

                # ---- W1 = Eh' x_aug  (d in col 64) ----
                w_ps = wpsp.tile([P, JT, 65], f32, tag="wps")
                for it in range(JT):
                    for jc in range(JT):
                        nc.tensor.matmul(
                            w_ps[:, it, :],
                            eh_sb[:, jc, it * P : (it + 1) * P],
                            xp_sb[:, jc, 0:65],
                            start=(jc == 0),
                            stop=(jc == JT - 1),
                        )

                # ---- per-node scalars ----
                d_sb = gsm.tile([P, JT], f32, tag="dsb")
                r_sb = gsm.tile([P, JT], f32, tag="rsb")
                rb_sb = gsm.tile([P, JT], f32, tag="rbsb")
                nc.vector.tensor_copy(d_sb[:], w_ps[:, :, 64])
                nc.vector.reciprocal(r_sb[:], d_sb[:])
                nc.vector.tensor_mul(rb_sb[:], s_ps[:], r_sb[:])

                wx_sb = gsm.tile([P, JT], f32, tag="wxsb")
                w1_sb = gsm.tile([P, JT], f32, tag="w1sb")
                w2_sb = gsm.tile([P, JT], f32, tag="w2sb")
                w3_sb = gsm.tile([P, JT], f32, tag="w3sb")
                nc.vector.tensor_scalar(wx_sb[:], rb_sb[:], Bx, Ax, mult, add)
                nc.vector.tensor_scalar(w1_sb[:], rb_sb[:], Bw[0], Aw[0], mult, add)
                nc.vector.tensor_scalar(w2_sb[:], rb_sb[:], Bw[1], Aw[1], mult, add)
                nc.vector.tensor_scalar(w3_sb[:], rb_sb[:], Bw[2], Aw[2], mult, add)

                # ---- V1 = r * W1 ----
                v_sb = [None] * 3
                v_sb[0] = gbig.tile([P, JT, C], f16, name="v1", tag="v1")
                rbc = r_sb[:].unsqueeze(2).broadcast_to([P, JT, C])
                nc.vector.tensor_mul(v_sb[0][:], w_ps[:, :, 0:C], rbc)

                # ---- steps 2,3 ----
                for k in (1, 2):
                    wk_ps = wpsp.tile([P, JT, 65], f32, tag="wps")
                    for it in range(JT):
                        for jc in range(JT):
                            nc.tensor.matmul(
                                wk_ps[:, it, 0:C],
                                eh_sb[:, jc, it * P : (it + 1) * P],
                                v_sb[k - 1][:, jc, :],
                                start=(jc == 0),
                                stop=(jc == JT - 1),
                            )
                    v_sb[k] = gbig.tile([P, JT, C], f16, name=f"v{k + 1}", tag=f"v{k + 1}")
                    nc.vector.tensor_mul(v_sb[k][:], wk_ps[:, :, 0:C], rbc)

                # ---- combine: OUT = wx*x + w1*V1 + w2*V2 + w3*V3 ----
                acc = gout.tile([P, JT, C], f16, tag="acc")
                tmp = gout.tile([P, JT, C], f16, tag="tmp")
                ob = gout.tile([P, JT, C], f32, tag="ob")
                wxb = wx_sb[:].unsqueeze(2).broadcast_to([P, JT, C])
                w1b = w1_sb[:].unsqueeze(2).broadcast_to([P, JT, C])
                w2b = w2_sb[:].unsqueeze(2).broadcast_to([P, JT, C])
                w3b = w3_sb[:].unsqueeze(2).broadcast_to([P, JT, C])
                nc.vector.tensor_mul(acc[:], xp_sb[:, :, 0:C], wxb)
                nc.vector.tensor_mul(tmp[:], v_sb[0][:], w1b)
                nc.vector.tensor_add(acc[:], acc[:], tmp[:])
                nc.vector.tensor_mul(tmp[:], v_sb[1][:], w2b)
                nc.vector.tensor_add(acc[:], acc[:], tmp[:])
                nc.vector.tensor_mul(tmp[:], v_sb[2][:], w3b)
                nc.vector.tensor_add(ob[:], acc[:], tmp[:])

                nc.sync.dma_start(out_d[t], ob[:])


def _host_pack(input, adj, a, temp, cheb):
    x = np.asarray(input, dtype=np.float32).transpose(0, 2, 1, 3)  # (B,T,N,C)
    adj = np.asarray(adj, dtype=np.float32)
    a = np.asarray(a, dtype=np.float32)
    temp = np.asarray(temp, dtype=np.float32)
    cheb = np.asarray(cheb, dtype=np.float32)

    a1, a2 = a[:C, 0], a[C:, 0]
    f1 = x @ a1  # (B,T,N)
    f2 = x @ a2  # (B,T,N)

    # --- scalar coefficient algebra (host, exact) ---
    coe = _sigmoid(temp)
    cc = _sigmoid(cheb)
    c0, c1, c2 = float(coe[0]), float(coe[1]), float(coe[2])
    g0, g1 = float(cc[0]), float(cc[1])
    gam = [1.0, g0, g0 * g1]
    # device rb = s/d (no 0.5): true rowsum = 0.5*rb -> fold 0.5 into B terms
    h = 0.5
    Ax = c2**3 + (1 - c2) * c0 * c1 * (c2**2 + c2 + 1)
    Bx = -(1 - c2) * c0 * (1 - c1) * (c2**2 * gam[0] + c2 * gam[1] + gam[2]) * h
    A1 = (1 - c2) * c2**2
    B1 = -(1 - c2) * c2**2 * (1 - c1) * gam[0] * h
    A2 = (1 - c2) * c2
    B2 = -(1 - c2) * c2 * (1 - c1) * gam[1] * h
    A3 = 1 - c2
    B3 = -(1 - c2) * (1 - c1) * gam[2] * h
    coefs = dict(Ax=Ax, Bx=Bx, A1=A1, B1=B1, A2=A2, B2=B2, A3=A3, B3=B3)

    # --- epack: rows [ones, ones, f2h, f2l, f1h, f1l, ones, ones] ---
    def hilo(v):
        hi = v.astype(np.float16)
        lo = (v - hi.astype(np.float32)).astype(np.float16)
        return hi, lo

    f1h, f1l = hilo(f1)
    f2h, f2l = hilo(f2)
    epack = np.empty((B, T, 8, 512), dtype=np.float16)
    epack[:, :, 0, :] = 1.0
    epack[:, :, 1, :] = 1.0
    epack[:, :, 2, :] = f2h
    epack[:, :, 3, :] = f2l
    epack[:, :, 4, :] = f1h
    epack[:, :, 5, :] = f1l
    epack[:, :, 6, :] = 1.0
    epack[:, :, 7, :] = 1.0

    # --- per-graph exp bias: shift max leaky(e) to 4 ---
    max_e = f1.max(axis=-1) + f2.max(axis=-1)  # (B,T)
    max_l = np.where(max_e > 0, max_e, LEAKY * max_e)
    bias_c = (4.0 - max_l).astype(np.float32)  # added inside exp

    # --- xp: x + ones col + bias col, node-major (p, jc) ---
    xr = x.reshape(B, T, JT, P, C)  # node = jc*128+p
    xp = np.empty((B, T, P, JT, 66), dtype=np.float16)
    xp[:, :, :, :, 0:C] = xr.transpose(0, 1, 3, 2, 4)
    xp[:, :, :, :, C] = 1.0
    xp[:, :, :, :, C + 1] = bias_c[:, :, None, None]

    # --- adjpack: AM = relu(adj)^T, Bm additive mask, (j,i) layout ---
    amT = np.maximum(adj, 0.0).T.astype(np.float16)  # [j,i]
    bmT = np.where(adj > 0.0, 0.0, MASK_NEG).T.astype(np.float16)
    adjpack = np.zeros((3, P, JT, 512), dtype=np.float16)
    adjpack[0] = amT.reshape(JT, P, 512).transpose(1, 0, 2)
    adjpack[1] = bmT.reshape(JT, P, 512).transpose(1, 0, 2)
    adjpack[2, :, 0, 0:128] = np.eye(P, dtype=np.float16)

    return epack, xp, adjpack, coefs


def kernel(input, h0, adj, a, temp, cheb):
    from concourse.bass_utils import run_bass_kernel_spmd

    epack, xp, adjpack, coefs = _host_pack(input, adj, a, temp, cheb)
    nc = _build_bass(coefs)

    in_maps = [
        {"epack": epack[b], "xp": xp[b], "adjpack": adjpack} for b in range(B)
    ]
    res = run_bass_kernel_spmd(nc, in_maps, core_ids=list(range(B)))
    outs = [res.results[b]["out"] for b in range(B)]  # (T,P,JT,C) each
    op = np.stack(outs, axis=0)  # (B,T,P,JT,C)
    # node = jc*128 + p -> (B, N, T, C)
    out = op.transpose(0, 3, 2, 1, 4).reshape(B, N, T, C)
    return np.ascontiguousarray(out.astype(np.float32))


if __name__ == "__main__":
    rng = np.random.default_rng(0)
    inp = rng.standard_normal((B, N, T, C), dtype=np.float32)
    h0 = rng.standard_normal((B, N, T, C), dtype=np.float32)
    adj = rng.standard_normal((N, N), dtype=np.float32)
    lim = 1.414 * np.sqrt(6.0 / (2 * C + 1))
    a = rng.uniform(-lim, lim, (2 * C, 1)).astype(np.float32)
    temp = np.zeros((HOP + 1,), np.float32)
    cheb = np.array([0.9 * 0.1**k for k in range(HOP + 1)], np.float32)
    out = kernel(inp, h0, adj, a, temp, cheb)
    print(out.shape, out.dtype, np.abs(out).mean())


# revision 13
# speedup vs baseline: 1.0786x; 1.0786x over previous
"""Trainium2 Bass kernel for nn_AdaptiveWaveletLayer.

Data-parallel over batch B across 8 NeuronCores (no collectives).
Per core: 12 graphs (t slices), each: masked-softmax attention over a
512x512 score matrix built from rank-1 terms, then 3 rounds of
U @ V message passing with all scalar coefficient algebra folded on host.

Device layout ((j,i) = transposed attention matrix, j on partitions):
  E'[j,i] = f1[i] + f2[j]      built by a K=4 fp16 hi/lo matmul into PSUM
  L = Lrelu(E', alpha=0.2)     ACT, PSUM->SBUF fp16
  Lm = L + Bmask (additive -30000 on masked entries)  DVE fp16 2x
  Eh = exp(Lm + bias_c)        ACT (per-graph range-shift bias), fp16
  G  = Eh * relu(adj)^T        DVE fp16 2x
  d[i] = sum_j Eh  (ones column folded into first matmul's rhs)
  s[i] = sum_j G   (ones-vector matmuls)
  W_k = Eh^T-contract matmuls; V_k = r * W_k  (r = 1/d)
  OUT = wx*x + w1*V1 + w2*V2 + w3*V3 (per-node affine weights in rowsum)
"""

import sys

if "/opt/trn_rl_repo" not in sys.path:
    sys.path.insert(0, "/opt/trn_rl_repo")

import numpy as np

B, N, T, C = 8, 512, 12, 64
P = 128
JT = N // P  # 4
HOP = 3
LEAKY = 0.2
MASK_NEG = -30000.0


def _sigmoid(x):
    return 1.0 / (1.0 + np.exp(-x))


def _build_bass(coefs, reps=None):
    """Build the single-core Bass graph. coefs: dict of python-float immediates."""
    from concourse import bacc, mybir
    from concourse.tile import TileContext

    f16 = mybir.dt.float16
    f32 = mybir.dt.float32
    Lrelu = mybir.ActivationFunctionType.Prelu
    Exp = mybir.ActivationFunctionType.Exp
    mult = mybir.AluOpType.mult
    add = mybir.AluOpType.add

    nc = bacc.Bacc()
    ep_d = nc.declare_dram_parameter("epack", [T, 8, 512], f16, isOutput=False)
    xp_d = nc.declare_dram_parameter("xp", [T, P, JT, 66], f16, isOutput=False)
    ad_d = nc.declare_dram_parameter("adjpack", [3, P, JT, 512], f16, isOutput=False)
    out_d = nc.declare_dram_parameter("out", [T, P, JT, C], f32, isOutput=True)

    Ax, Bx = coefs["Ax"], coefs["Bx"]
    Aw = [coefs["A1"], coefs["A2"], coefs["A3"]]
    Bw = [coefs["B1"], coefs["B2"], coefs["B3"]]

    with TileContext(nc) as tc:
        with (
            tc.tile_pool(name="const", bufs=1) as constp,
            tc.tile_pool(name="gin", bufs=3) as gin,
            tc.tile_pool(name="gbig", bufs=2) as gbig,
            tc.tile_pool(name="gsm", bufs=2) as gsm,
            tc.tile_pool(name="gout", bufs=2) as gout,
            tc.tile_pool(name="eps", bufs=1, space="PSUM") as epsp,
            tc.tile_pool(name="wps", bufs=2, space="PSUM") as wpsp,
            tc.tile_pool(name="sps", bufs=2, space="PSUM") as spsp,
        ):
            # ---- shared constants ----
            am_sb = constp.tile([P, JT, 512], f16, tag="am")
            bm_sb = constp.tile([P, JT, 512], f16, tag="bm")
            id_sb = constp.tile([P, 128], f16, tag="idm")
            ones_sb = constp.tile([P, 1], f16, tag="ones")
            nc.sync.dma_start(am_sb[:], ad_d[0])
            nc.sync.dma_start(bm_sb[:], ad_d[1])
            nc.sync.dma_start(id_sb[:], ad_d[2, :, 0, 0:128])
            nc.vector.memset(ones_sb[:], 1.0)

            import contextlib

            loop_cm = tc.For_i(0, reps, 1) if reps else contextlib.nullcontext()
            with loop_cm:
                _body_graphs(nc, tc, locals())

    nc.finalize()
    return nc


def _body_graphs(nc, tc, env):
    from concourse import mybir

    f16 = mybir.dt.float16
    f32 = mybir.dt.float32
    Lrelu = mybir.ActivationFunctionType.Prelu
    Exp = mybir.ActivationFunctionType.Exp
    mult = mybir.AluOpType.mult
    add = mybir.AluOpType.add
    gin, gbig, gsm, gout = env["gin"], env["gbig"], env["gsm"], env["gout"]
    epsp, wpsp, spsp = env["epsp"], env["wpsp"], env["spsp"]
    am_sb, bm_sb, ones_sb, id_sb = env["am_sb"], env["bm_sb"], env["ones_sb"], env["id_sb"]
    ep_d, xp_d, out_d = env["ep_d"], env["xp_d"], env["out_d"]
    Ax, Bx, Aw, Bw = env["Ax"], env["Bx"], env["Aw"], env["Bw"]
    if True:
            for t in range(T):
                # ---- per-graph inputs ----
                elhs = gin.tile([4, 512], f16, tag="elhs")
                erhs = gin.tile([4, 512], f16, tag="erhs")
                xp_sb = gin.tile([P, JT, 66], f16, tag="xp")
                nc.sync.dma_start(elhs[:], ep_d[t, 0:4, :])
                nc.sync.dma_start(erhs[:], ep_d[t, 4:8, :])
                nc.sync.dma_start(xp_sb[:], xp_d[t])

                # ---- E' = f1[i] + f2[j]  (K=4 fp16 hi/lo) ----
                e_ps = epsp.tile([P, JT, 512], f32, tag="eps")
                for jt in range(JT):
                    nc.tensor.matmul(
                        e_ps[:, jt, :],
                        elhs[:, jt * P : (jt + 1) * P],
                        erhs[:],
                        start=True,
                        stop=False,
                    )
                    nc.tensor.matmul(
                        e_ps[:, jt, :],
                        id_sb[:],
                        bm_sb[:, jt, :],
                        start=False,
                        stop=True,
                    )

                # ---- leaky relu (mask already folded into E) ----
                l_sb = gbig.tile([P, JT, 512], f16, tag="lsb")
                nc.scalar.activation(l_sb[:], e_ps[:], Lrelu, alpha=LEAKY)

                # ---- exp with per-graph range shift ----
                eh_sb = gbig.tile([P, JT, 512], f16, tag="ehsb")
                nc.scalar.activation(
                    eh_sb[:], l_sb[:], Exp, bias=xp_sb[:, 0, 65:66], scale=1.0
                )

                # ---- G = Eh * relu(adj)^T ----
                g_sb = gbig.tile([P, JT, 512], f16, tag="gsb")
                nc.vector.tensor_mul(g_sb[:], eh_sb[:], am_sb[:])

                # ---- s[i] = sum_j G ----
                s_ps = spsp.tile([P, JT], f32, tag="sps")
                for it in range(JT):
                    for jc in range(JT):
                        nc.tensor.matmul(
                            s_ps[:, it : it + 1],
                            g_sb[:, jc, it * P : (it + 1) * P],
                            ones_sb[:],
                            start=(jc == 0),
                            stop=(jc == JT - 1),
                        )

                # ---- W1 = Eh' x_aug  (d in col 64) ----
                w_ps = wpsp.tile([P, JT, 65], f32, tag="wps")
                for it in range(JT):
                    for jc in range(JT):
                        nc.tensor.matmul(
                            w_ps[:, it, :],
                            eh_sb[:, jc, it * P : (it + 1) * P],
                            xp_sb[:, jc, 0:65],
                            start=(jc == 0),
                            stop=(jc == JT - 1),
                        )

                # ---- per-node scalars ----
                d_sb = gsm.tile([P, JT], f32, tag="dsb")
                r_sb = gsm.tile([P, JT], f32, tag="rsb")
                rb_sb = gsm.tile([P, JT], f32, tag="rbsb")
                nc.vector.tensor_copy(d_sb[:], w_ps[:, :, 64])
                nc.vector.reciprocal(r_sb[:], d_sb[:])
                nc.vector.tensor_mul(rb_sb[:], s_ps[:], r_sb[:])

                wx_sb = gsm.tile([P, JT], f32, tag="wxsb")
                w1_sb = gsm.tile([P, JT], f32, tag="w1sb")
                w2_sb = gsm.tile([P, JT], f32, tag="w2sb")
                w3_sb = gsm.tile([P, JT], f32, tag="w3sb")
                nc.vector.tensor_scalar(wx_sb[:], rb_sb[:], Bx, Ax, mult, add)
                nc.vector.tensor_scalar(w1_sb[:], rb_sb[:], Bw[0], Aw[0], mult, add)
                nc.vector.tensor_scalar(w2_sb[:], rb_sb[:], Bw[1], Aw[1], mult, add)
                nc.vector.tensor_scalar(w3_sb[:], rb_sb[:], Bw[2], Aw[2], mult, add)

                # ---- V1 = r * W1 ----
                v_sb = [None] * 3
                v_sb[0] = gbig.tile([P, JT, C], f16, name="v1", tag="v1")
                rbc = r_sb[:].unsqueeze(2).broadcast_to([P, JT, C])
                nc.vector.tensor_mul(v_sb[0][:], w_ps[:, :, 0:C], rbc)

                # ---- steps 2,3 ----
                for k in (1, 2):
                    wk_ps = wpsp.tile([P, JT, 65], f32, tag="wps")
                    for it in range(JT):
                        for jc in range(JT):
                            nc.tensor.matmul(
                                wk_ps[:, it, 0:C],
                                eh_sb[:, jc, it * P : (it + 1) * P],
                                v_sb[k - 1][:, jc, :],
                                start=(jc == 0),
                                stop=(jc == JT - 1),
                            )
                    v_sb[k] = gbig.tile([P, JT, C], f16, name=f"v{k + 1}", tag=f"v{k + 1}")
                    nc.vector.tensor_mul(v_sb[k][:], wk_ps[:, :, 0:C], rbc)

                # ---- combine: OUT = wx*x + w1*V1 + w2*V2 + w3*V3 ----
                acc = gout.tile([P, JT, C], f16, tag="acc")
                tmp = gout.tile([P, JT, C], f16, tag="tmp")
                ob = gout.tile([P, JT, C], f32, tag="ob")
                wxb = wx_sb[:].unsqueeze(2).broadcast_to([P, JT, C])
                w1b = w1_sb[:].unsqueeze(2).broadcast_to([P, JT, C])
                w2b = w2_sb[:].unsqueeze(2).broadcast_to([P, JT, C])
                w3b = w3_sb[:].unsqueeze(2).broadcast_to([P, JT, C])
                nc.vector.tensor_mul(acc[:], xp_sb[:, :, 0:C], wxb)
                nc.vector.tensor_mul(tmp[:], v_sb[0][:], w1b)
                nc.vector.tensor_add(acc[:], acc[:], tmp[:])
                nc.vector.tensor_mul(tmp[:], v_sb[1][:], w2b)
                nc.vector.tensor_add(acc[:], acc[:], tmp[:])
                nc.vector.tensor_mul(tmp[:], v_sb[2][:], w3b)
                nc.vector.tensor_add(ob[:], acc[:], tmp[:])

                nc.sync.dma_start(out_d[t], ob[:])


def _host_pack(input, adj, a, temp, cheb):
    x = np.asarray(input, dtype=np.float32).transpose(0, 2, 1, 3)  # (B,T,N,C)
    adj = np.asarray(adj, dtype=np.float32)
    a = np.asarray(a, dtype=np.float32)
    temp = np.asarray(temp, dtype=np.float32)
    cheb = np.asarray(cheb, dtype=np.float32)

    a1, a2 = a[:C, 0], a[C:, 0]
    f1 = x @ a1  # (B,T,N)
    f2 = x @ a2  # (B,T,N)

    # --- scalar coefficient algebra (host, exact) ---
    coe = _sigmoid(temp)
    cc = _sigmoid(cheb)
    c0, c1, c2 = float(coe[0]), float(coe[1]), float(coe[2])
    g0, g1 = float(cc[0]), float(cc[1])
    gam = [1.0, g0, g0 * g1]
    # device rb = s/d (no 0.5): true rowsum = 0.5*rb -> fold 0.5 into B terms
    h = 0.5
    Ax = c2**3 + (1 - c2) * c0 * c1 * (c2**2 + c2 + 1)
    Bx = -(1 - c2) * c0 * (1 - c1) * (c2**2 * gam[0] + c2 * gam[1] + gam[2]) * h
    A1 = (1 - c2) * c2**2
    B1 = -(1 - c2) * c2**2 * (1 - c1) * gam[0] * h
    A2 = (1 - c2) * c2
    B2 = -(1 - c2) * c2 * (1 - c1) * gam[1] * h
    A3 = 1 - c2
    B3 = -(1 - c2) * (1 - c1) * gam[2] * h
    coefs = dict(Ax=Ax, Bx=Bx, A1=A1, B1=B1, A2=A2, B2=B2, A3=A3, B3=B3)

    # --- epack: rows [ones, ones, f2h, f2l, f1h, f1l, ones, ones] ---
    def hilo(v):
        hi = v.astype(np.float16)
        lo = (v - hi.astype(np.float32)).astype(np.float16)
        return hi, lo

    f1h, f1l = hilo(f1)
    f2h, f2l = hilo(f2)
    epack = np.empty((B, T, 8, 512), dtype=np.float16)
    epack[:, :, 0, :] = 1.0
    epack[:, :, 1, :] = 1.0
    epack[:, :, 2, :] = f2h
    epack[:, :, 3, :] = f2l
    epack[:, :, 4, :] = f1h
    epack[:, :, 5, :] = f1l
    epack[:, :, 6, :] = 1.0
    epack[:, :, 7, :] = 1.0

    # --- per-graph exp bias: shift max leaky(e) to 4 ---
    max_e = f1.max(axis=-1) + f2.max(axis=-1)  # (B,T)
    max_l = np.where(max_e > 0, max_e, LEAKY * max_e)
    bias_c = (4.0 - max_l).astype(np.float32)  # added inside exp

    # --- xp: x + ones col + bias col, node-major (p, jc) ---
    xr = x.reshape(B, T, JT, P, C)  # node = jc*128+p
    xp = np.empty((B, T, P, JT, 66), dtype=np.float16)
    xp[:, :, :, :, 0:C] = xr.transpose(0, 1, 3, 2, 4)
    xp[:, :, :, :, C] = 1.0
    xp[:, :, :, :, C + 1] = bias_c[:, :, None, None]

    # --- adjpack: AM = relu(adj)^T, Bm additive mask, (j,i) layout ---
    amT = np.maximum(adj, 0.0).T.astype(np.float16)  # [j,i]
    bmT = np.where(adj > 0.0, 0.0, MASK_NEG).T.astype(np.float16)
    adjpack = np.zeros((3, P, JT, 512), dtype=np.float16)
    adjpack[0] = amT.reshape(JT, P, 512).transpose(1, 0, 2)
    adjpack[1] = bmT.reshape(JT, P, 512).transpose(1, 0, 2)
    adjpack[2, :, 0, 0:128] = np.eye(P, dtype=np.float16)

    return epack, xp, adjpack, coefs


def kernel(input, h0, adj, a, temp, cheb):
    from concourse.bass_utils import run_bass_kernel_spmd

    epack, xp, adjpack, coefs = _host_pack(input, adj, a, temp, cheb)
    nc = _build_bass(coefs)

    in_maps = [
        {"epack": epack[b], "xp": xp[b], "adjpack": adjpack} for b in range(B)
    ]
    res = run_bass_kernel_spmd(nc, in_maps, core_ids=list(range(B)))
    outs = [res.results[b]["out"] for b in range(B)]  # (T,P,JT,C) each
    op = np.stack(outs, axis=0)  # (B,T,P,JT,C)
    # node = jc*128 + p -> (B, N, T, C)
    out = op.transpose(0, 3, 2, 1, 4).reshape(B, N, T, C)
    return np.ascontiguousarray(out.astype(np.float32))


if __name__ == "__main__":
    rng = np.random.default_rng(0)
    inp = rng.standard_normal((B, N, T, C), dtype=np.float32)
    h0 = rng.standard_normal((B, N, T, C), dtype=np.float32)
    adj = rng.standard_normal((N, N), dtype=np.float32)
    lim = 1.414 * np.sqrt(6.0 / (2 * C + 1))
    a = rng.uniform(-lim, lim, (2 * C, 1)).astype(np.float32)
    temp = np.zeros((HOP + 1,), np.float32)
    cheb = np.array([0.9 * 0.1**k for k in range(HOP + 1)], np.float32)
    out = kernel(inp, h0, adj, a, temp, cheb)
    print(out.shape, out.dtype, np.abs(out).mean())


# revision 14
# speedup vs baseline: 2.7756x; 2.5734x over previous
"""Trainium2 Bass kernel for nn_AdaptiveWaveletLayer.

Data-parallel over batch B across 8 NeuronCores (no collectives).
Per core: 12 graphs (t slices), each: masked-softmax attention over a
512x512 score matrix built from rank-1 terms, then 3 rounds of
U @ V message passing with all scalar coefficient algebra folded on host.

Device layout ((j,i) = transposed attention matrix, j on partitions):
  E'[j,i] = f1[i] + f2[j] + Bmask   K=4 fp16 hi/lo matmul + identity-matmul
                                    mask accumulation into PSUM
  L  = leaky(E') via Prelu(alpha=0.2)   ACT, PSUM->SBUF fp16
  Eh = exp(L + bias_c)              ACT (per-graph range-shift bias), fp16
  G  = Eh * relu(adj)^T             DVE fp16 2x
  d[i] = sum_j Eh  (ones column folded into first matmul's rhs)
  s[i] = sum_j G   (ones-vector matmuls)
  W_k = Eh'-contract matmuls; V_k = r * W_k  (r = 1/d)
  OUT = wx*x + w1*V1 + w2*V2 + w3*V3 (per-node affine weights in rowsum)
All inputs are DMA'd once upfront (partition-contiguous host layouts).
"""

import sys

if "/opt/trn_rl_repo" not in sys.path:
    sys.path.insert(0, "/opt/trn_rl_repo")

import numpy as np

B, N, T, C = 8, 512, 12, 64
P = 128
JT = N // P  # 4
HOP = 3
LEAKY = 0.2
MASK_NEG = -30000.0


def _sigmoid(x):
    return 1.0 / (1.0 + np.exp(-x))


def _build_bass(coefs, reps=None):
    """Build the single-core Bass graph. coefs: dict of python-float immediates."""
    import contextlib

    from concourse import bacc, mybir
    from concourse.tile import TileContext

    f16 = mybir.dt.float16
    f32 = mybir.dt.float32

    nc = bacc.Bacc()
    ep_d = nc.declare_dram_parameter("epack", [8, T, 512], f16, isOutput=False)
    xp_d = nc.declare_dram_parameter("xp", [P, T, JT, 66], f16, isOutput=False)
    ad_d = nc.declare_dram_parameter("adjpack", [3, P, JT, 512], f16, isOutput=False)
    out_d = nc.declare_dram_parameter("out", [T, P, JT, C], f32, isOutput=True)

    with TileContext(nc) as tc:
        with (
            tc.tile_pool(name="const", bufs=1) as constp,
            tc.tile_pool(name="gbig", bufs=3) as gbig,
            tc.tile_pool(name="gsm", bufs=3) as gsm,
            tc.tile_pool(name="gout", bufs=3) as gout,
            tc.tile_pool(name="eps", bufs=1, space="PSUM") as epsp,
            tc.tile_pool(name="wps", bufs=2, space="PSUM") as wpsp,
            tc.tile_pool(name="sps", bufs=2, space="PSUM") as spsp,
        ):
            # ---- constants + all inputs, loaded once ----
            am_sb = constp.tile([P, JT, 512], f16, tag="am")
            bm_sb = constp.tile([P, JT, 512], f16, tag="bm")
            id_sb = constp.tile([P, 128], f16, tag="idm")
            ones_sb = constp.tile([P, 1], f16, tag="ones")
            elhs = constp.tile([4, T, 512], f16, tag="elhs")
            erhs = constp.tile([4, T, 512], f16, tag="erhs")
            xp_sb = constp.tile([P, T, JT, 66], f16, tag="xp")
            nc.sync.dma_start(am_sb[:], ad_d[0])
            nc.sync.dma_start(bm_sb[:], ad_d[1])
            nc.sync.dma_start(id_sb[:], ad_d[2, :, 0, 0:128])
            nc.sync.dma_start(elhs[:], ep_d[0:4])
            nc.sync.dma_start(erhs[:], ep_d[4:8])
            nc.sync.dma_start(xp_sb[:], xp_d[:])
            nc.vector.memset(ones_sb[:], 1.0)

            env = dict(
                gbig=gbig, gsm=gsm, gout=gout, epsp=epsp, wpsp=wpsp, spsp=spsp,
                am_sb=am_sb, bm_sb=bm_sb, id_sb=id_sb, ones_sb=ones_sb,
                elhs=elhs, erhs=erhs, xp_sb=xp_sb, out_d=out_d, coefs=coefs,
            )
            loop_cm = tc.For_i(0, reps, 1) if reps else contextlib.nullcontext()
            with loop_cm:
                _body_graphs(nc, env)

    nc.finalize()
    return nc


def _body_graphs(nc, env):
    from concourse import mybir

    f16 = mybir.dt.float16
    f32 = mybir.dt.float32
    Prelu = mybir.ActivationFunctionType.Prelu
    Exp = mybir.ActivationFunctionType.Exp
    mult = mybir.AluOpType.mult
    add = mybir.AluOpType.add
    gbig, gsm, gout = env["gbig"], env["gsm"], env["gout"]
    epsp, wpsp, spsp = env["epsp"], env["wpsp"], env["spsp"]
    am_sb, bm_sb, id_sb, ones_sb = (
        env["am_sb"], env["bm_sb"], env["id_sb"], env["ones_sb"],
    )
    elhs, erhs, xp_sb, out_d = env["elhs"], env["erhs"], env["xp_sb"], env["out_d"]
    coefs = env["coefs"]
    Ax, Bx = coefs["Ax"], coefs["Bx"]
    Aw = [coefs["A1"], coefs["A2"], coefs["A3"]]
    Bw = [coefs["B1"], coefs["B2"], coefs["B3"]]

    for t in range(T):
        xg = xp_sb[:, t]  # (P, JT, 66)

        # ---- E'[j,i] = f1[i] + f2[j] + Bmask  (K=4 fp16 hi/lo + id matmul) ----
        e_ps = epsp.tile([P, JT, 512], f32, name="e_ps", tag="eps")
        for jt in range(JT):
            nc.tensor.matmul(
                e_ps[:, jt, :],
                elhs[:, t, jt * P : (jt + 1) * P],
                erhs[:, t, :],
                start=True,
                stop=False,
            )
            nc.tensor.matmul(
                e_ps[:, jt, :], id_sb[:], bm_sb[:, jt, :], start=False, stop=True
            )

        # ---- leaky relu (mask already folded into E) ----
        l_sb = gbig.tile([P, JT, 512], f16, name="l_sb", tag="lsb")
        nc.scalar.activation(l_sb[:], e_ps[:], Prelu, alpha=LEAKY)

        # ---- exp with per-graph range shift ----
        eh_sb = gbig.tile([P, JT, 512], f16, name="eh_sb", tag="ehsb")
        nc.scalar.activation(eh_sb[:], l_sb[:], Exp, bias=xg[:, 0, 65:66], scale=1.0)

        # ---- G = Eh * relu(adj)^T ----
        g_sb = gbig.tile([P, JT, 512], f16, name="g_sb", tag="gsb")
        nc.vector.tensor_mul(g_sb[:], eh_sb[:], am_sb[:])

        # ---- s[i] = sum_j G ----
        s_ps = spsp.tile([P, JT], f32, name="s_ps", tag="sps")
        for it in range(JT):
            for jc in range(JT):
                nc.tensor.matmul(
                    s_ps[:, it : it + 1],
                    g_sb[:, jc, it * P : (it + 1) * P],
                    ones_sb[:],
                    start=(jc == 0),
                    stop=(jc == JT - 1),
                )

        # ---- W1 = Eh' x_aug  (d in col 64) ----
        w_ps = wpsp.tile([P, JT, 65], f32, name="w_ps", tag="wps")
        for it in range(JT):
            for jc in range(JT):
                nc.tensor.matmul(
                    w_ps[:, it, :],
                    eh_sb[:, jc, it * P : (it + 1) * P],
                    xg[:, jc, 0:65],
                    start=(jc == 0),
                    stop=(jc == JT - 1),
                )

        # ---- per-node scalars ----
        d_sb = gsm.tile([P, JT], f32, name="d_sb", tag="dsb")
        r_sb = gsm.tile([P, JT], f32, name="r_sb", tag="rsb")
        rb_sb = gsm.tile([P, JT], f32, name="rb_sb", tag="rbsb")
        nc.vector.tensor_copy(d_sb[:], w_ps[:, :, 64])
        nc.vector.reciprocal(r_sb[:], d_sb[:])
        nc.vector.tensor_mul(rb_sb[:], s_ps[:], r_sb[:])

        wx_sb = gsm.tile([P, JT], f32, name="wx_sb", tag="wxsb")
        w1_sb = gsm.tile([P, JT], f32, name="w1_sb", tag="w1sb")
        w2_sb = gsm.tile([P, JT], f32, name="w2_sb", tag="w2sb")
        w3_sb = gsm.tile([P, JT], f32, name="w3_sb", tag="w3sb")
        nc.vector.tensor_scalar(wx_sb[:], rb_sb[:], Bx, Ax, mult, add)
        nc.vector.tensor_scalar(w1_sb[:], rb_sb[:], Bw[0], Aw[0], mult, add)
        nc.vector.tensor_scalar(w2_sb[:], rb_sb[:], Bw[1], Aw[1], mult, add)
        nc.vector.tensor_scalar(w3_sb[:], rb_sb[:], Bw[2], Aw[2], mult, add)

        # ---- V1 = r * W1 ----
        v_sb = [None] * 3
        v_sb[0] = gbig.tile([P, JT, C], f16, name="v1", tag="v1")
        rbc = r_sb[:].unsqueeze(2).broadcast_to([P, JT, C])
        nc.vector.tensor_mul(v_sb[0][:], w_ps[:, :, 0:C], rbc)

        # ---- steps 2,3 ----
        for k in (1, 2):
            wk_ps = wpsp.tile([P, JT, 65], f32, name="wk_ps", tag="wps")
            for it in range(JT):
                for jc in range(JT):
                    nc.tensor.matmul(
                        wk_ps[:, it, 0:C],
                        eh_sb[:, jc, it * P : (it + 1) * P],
                        v_sb[k - 1][:, jc, :],
                        start=(jc == 0),
                        stop=(jc == JT - 1),
                    )
            v_sb[k] = gbig.tile([P, JT, C], f16, name=f"v{k + 1}", tag=f"v{k + 1}")
            nc.vector.tensor_mul(v_sb[k][:], wk_ps[:, :, 0:C], rbc)

        # ---- combine: OUT = wx*x + w1*V1 + w2*V2 + w3*V3 ----
        acc = gout.tile([P, JT, C], f16, name="acc", tag="acc")
        tmp = gout.tile([P, JT, C], f16, name="tmp", tag="tmp")
        ob = gout.tile([P, JT, C], f32, name="ob", tag="ob")
        wxb = wx_sb[:].unsqueeze(2).broadcast_to([P, JT, C])
        w1b = w1_sb[:].unsqueeze(2).broadcast_to([P, JT, C])
        w2b = w2_sb[:].unsqueeze(2).broadcast_to([P, JT, C])
        w3b = w3_sb[:].unsqueeze(2).broadcast_to([P, JT, C])
        nc.vector.tensor_mul(acc[:], xg[:, :, 0:C], wxb)
        nc.vector.tensor_mul(tmp[:], v_sb[0][:], w1b)
        nc.vector.tensor_add(acc[:], acc[:], tmp[:])
        nc.vector.tensor_mul(tmp[:], v_sb[1][:], w2b)
        nc.vector.tensor_add(acc[:], acc[:], tmp[:])
        nc.vector.tensor_mul(tmp[:], v_sb[2][:], w3b)
        nc.vector.tensor_add(ob[:], acc[:], tmp[:])

        nc.scalar.dma_start(out_d[t], ob[:])


def _host_pack(input, adj, a, temp, cheb):
    x = np.asarray(input, dtype=np.float32).transpose(0, 2, 1, 3)  # (B,T,N,C)
    adj = np.asarray(adj, dtype=np.float32)
    a = np.asarray(a, dtype=np.float32)
    temp = np.asarray(temp, dtype=np.float32)
    cheb = np.asarray(cheb, dtype=np.float32)

    a1, a2 = a[:C, 0], a[C:, 0]
    f1 = x @ a1  # (B,T,N)
    f2 = x @ a2  # (B,T,N)

    # --- scalar coefficient algebra (host, exact) ---
    coe = _sigmoid(temp)
    cc = _sigmoid(cheb)
    c0, c1, c2 = float(coe[0]), float(coe[1]), float(coe[2])
    g0, g1 = float(cc[0]), float(cc[1])
    gam = [1.0, g0, g0 * g1]
    h = 0.5  # device rb = s/d (no 0.5): fold into B terms
    Ax = c2**3 + (1 - c2) * c0 * c1 * (c2**2 + c2 + 1)
    Bx = -(1 - c2) * c0 * (1 - c1) * (c2**2 * gam[0] + c2 * gam[1] + gam[2]) * h
    A1 = (1 - c2) * c2**2
    B1 = -(1 - c2) * c2**2 * (1 - c1) * gam[0] * h
    A2 = (1 - c2) * c2
    B2 = -(1 - c2) * c2 * (1 - c1) * gam[1] * h
    A3 = 1 - c2
    B3 = -(1 - c2) * (1 - c1) * gam[2] * h
    coefs = dict(Ax=Ax, Bx=Bx, A1=A1, B1=B1, A2=A2, B2=B2, A3=A3, B3=B3)

    # --- epack rows: [ones, ones, f2h, f2l | f1h, f1l, ones, ones] ---
    def hilo(v):
        hi = v.astype(np.float16)
        lo = (v - hi.astype(np.float32)).astype(np.float16)
        return hi, lo

    f1h, f1l = hilo(f1)
    f2h, f2l = hilo(f2)
    epack = np.empty((B, 8, T, 512), dtype=np.float16)
    epack[:, 0] = 1.0
    epack[:, 1] = 1.0
    epack[:, 2] = f2h.transpose(0, 1, 2)
    epack[:, 3] = f2l
    epack[:, 4] = f1h
    epack[:, 5] = f1l
    epack[:, 6] = 1.0
    epack[:, 7] = 1.0

    # --- per-graph exp bias: shift max leaky(e) to 4 ---
    max_e = f1.max(axis=-1) + f2.max(axis=-1)  # (B,T)
    max_l = np.where(max_e > 0, max_e, LEAKY * max_e)
    bias_c = (4.0 - max_l).astype(np.float32)

    # --- xp: x + ones col + bias col, (p, t, jc, c) partition-contiguous ---
    xr = x.reshape(B, T, JT, P, C)  # node = jc*128+p
    xp = np.empty((B, P, T, JT, 66), dtype=np.float16)
    xp[:, :, :, :, 0:C] = xr.transpose(0, 3, 1, 2, 4)
    xp[:, :, :, :, C] = 1.0
    xp[:, :, :, :, C + 1] = bias_c[:, None, :, None]

    # --- adjpack: AM = relu(adj)^T, Bm additive mask, identity plane ---
    amT = np.maximum(adj, 0.0).T.astype(np.float16)  # [j,i]
    bmT = np.where(adj > 0.0, 0.0, MASK_NEG).T.astype(np.float16)
    adjpack = np.zeros((3, P, JT, 512), dtype=np.float16)
    adjpack[0] = amT.reshape(JT, P, 512).transpose(1, 0, 2)
    adjpack[1] = bmT.reshape(JT, P, 512).transpose(1, 0, 2)
    adjpack[2, :, 0, 0:128] = np.eye(P, dtype=np.float16)

    return epack, xp, adjpack, coefs


def kernel(input, h0, adj, a, temp, cheb):
    from concourse.bass_utils import run_bass_kernel_spmd

    epack, xp, adjpack, coefs = _host_pack(input, adj, a, temp, cheb)
    nc = _build_bass(coefs)

    in_maps = [
        {"epack": epack[b], "xp": xp[b], "adjpack": adjpack} for b in range(B)
    ]
    res = run_bass_kernel_spmd(nc, in_maps, core_ids=list(range(B)))
    outs = [res.results[b]["out"] for b in range(B)]  # (T,P,JT,C) each
    op = np.stack(outs, axis=0)  # (B,T,P,JT,C)
    out = op.transpose(0, 3, 2, 1, 4).reshape(B, N, T, C)
    return np.ascontiguousarray(out.astype(np.float32))


if __name__ == "__main__":
    rng = np.random.default_rng(0)
    inp = rng.standard_normal((B, N, T, C), dtype=np.float32)
    h0 = rng.standard_normal((B, N, T, C), dtype=np.float32)
    adj = rng.standard_normal((N, N), dtype=np.float32)
    lim = 1.414 * np.sqrt(6.0 / (2 * C + 1))
    a = rng.uniform(-lim, lim, (2 * C, 1)).astype(np.float32)
    temp = np.zeros((HOP + 1,), np.float32)
    cheb = np.array([0.9 * 0.1**k for k in range(HOP + 1)], np.float32)
    out = kernel(inp, h0, adj, a, temp, cheb)
    print(out.shape, out.dtype, np.abs(out).mean())
